# revision 1
# baseline (speedup 1.0000x reference)
"""Variable-length average pooling (prefix mean over seq axis) on 8 trn2 cores.

Strategy (pure data parallelism over batch):
  - eff_len[b] = lengths[b] if >0 else L.  pooled[b] = sum_{l<eff} x[b,l,:] / eff.
  - Sort batches by eff_len desc, snake-assign 16 per core so per-core work and
    per-slot length profiles are balanced across cores (~0.8% imbalance).
  - One SPMD Bass program shared by all 8 cores: slot j processes
    ceil(max_core_len_j/128) L-chunks of [rows<=128, 2048]; rows beyond a
    core's own length are zeroed by the per-core mask weights, so only the
    slot-max structure is baked into the program (+5% extra DMA vs ideal).
  - Chunks are fetched full-width (128 rows) in 2 MB pairs, alternating the
    two HWDGE rings (SP/ACT): partial-partition DMAs pile onto the low SDMA
    engines (measured +60%), and a single ring caps at ~318 GB/s vs ~390 for
    the pair. Invalid rows cost bytes but spread evenly; masks zero them.
  - fp32 moving operands run the PE at 1/4 rate, which would make PE the
    bottleneck (~293us busy vs ~200us DMA), so the reduction is split:
      * "uniform" chunks (all 128 rows valid on every core, i.e.
        128*(k+1) <= min_core_len) are summed on the VectorE into an SBUF
        accumulator (tensor_tensor add, full fp32), then reduced across
        partitions by one PE matmul against a 1/len column.
      * ragged chunks go straight to the PE as
        psum[1,512] += maskcol[128,1].T @ tile[128,512],
        maskcol[p] = (128k+p < eff)/eff (scale folded in).
  - PSUM halves -> SBUF via VectorE copy (ACT issues DMAs; a copy queued
    behind a stalled DMA issue would delay the PSUM release) -> DMA out.
"""

import os

import numpy as np

import concourse.bacc as bacc
import concourse.mybir as mybir
from concourse.tile import TileContext
from concourse.bass_utils import run_bass_kernel_spmd

B, L, D = 128, 1024, 2048
NCORES = 8
SLOTS = B // NCORES  # 16
PCHUNK = 128         # L-rows per chunk (partition dim of the tile)
MAXK = L // PCHUNK   # 8
NTILE = 512          # matmul moving free dim (one PSUM bank of fp32)
MCOLS = SLOTS * MAXK + SLOTS  # mask columns + per-slot 1/len columns

TILE_BUFS = int(os.environ.get("TILE_BUFS", "6"))

LAST_RESULTS = None  # BassKernelResults of the most recent device run


def _plan(eff):
    """Snake-assign sorted batches to cores.

    Returns (cores[c][s] -> batch idx, slot_rows[s] -> per-chunk row counts,
    slot_uniform[s] -> #leading chunks full on every core)."""
    order = np.argsort(-eff, kind="stable")
    cores = [[] for _ in range(NCORES)]
    for i, idx in enumerate(order):
        blk, pos = divmod(i, NCORES)
        c = pos if blk % 2 == 0 else NCORES - 1 - pos
        cores[c].append(int(idx))
    slot_rows, slot_uniform = [], []
    for s in range(SLOTS):
        lens = [int(eff[cores[c][s]]) for c in range(NCORES)]
        m, mn = max(lens), min(lens)
        nk = -(-m // PCHUNK)
        slot_rows.append(tuple(min(PCHUNK, m - PCHUNK * k) for k in range(nk)))
        slot_uniform.append(mn // PCHUNK)
    return cores, tuple(slot_rows), tuple(slot_uniform)


_PROGRAM_CACHE = {}


def _build_program(slot_rows, slot_uniform):
    # Bacc (not raw Bass): its compile pass splits multi-sem waits and moves
    # matmul waits onto ldweights — walrus allows only 1 wait per instruction.
    nc = bacc.Bacc(None, target_bir_lowering=False)
    f32 = mybir.dt.float32
    feat = nc.dram_tensor("features", [SLOTS, L, D], f32, kind="ExternalInput")
    maskt = nc.dram_tensor("maskt", [PCHUNK, MCOLS], f32, kind="ExternalInput")
    out = nc.dram_tensor("out", [SLOTS, D], f32, kind="ExternalOutput")

    with TileContext(nc) as tc:
        with (
            tc.tile_pool(name="mask", bufs=1) as mpool,
            tc.tile_pool(name="tiles", bufs=TILE_BUFS) as tpool,
            tc.tile_pool(name="accs", bufs=2) as apool,
            tc.tile_pool(name="psum", bufs=4, space="PSUM") as ppool,
            tc.tile_pool(name="outs", bufs=3) as opool,
        ):
            mask_tile = mpool.tile([PCHUNK, MCOLS], f32)
            nc.sync.dma_start(out=mask_tile[:], in_=maskt[:])
            # Alternate the two HWDGE rings (SP + ACT) for the big loads:
            # measured 318 -> ~390 GB/s vs a single ring.
            dma_engines = [nc.sync, nc.scalar]
            n_dma = 0
            for s in range(SLOTS):
                rows_list = slot_rows[s]
                nk = len(rows_list)
                nu = slot_uniform[s]
                psum_a = ppool.tile([1, D // 2], f32, name="psum_a", tag="ps")
                psum_b = ppool.tile([1, D // 2], f32, name="psum_b", tag="ps")
                psum_half = [psum_a, psum_a, psum_b, psum_b]
                acc = (
                    apool.tile([PCHUNK, D], f32, name="acc", tag="acc")
                    if nu > 0
                    else None
                )

                # Load L-chunks in 2 MB pairs [128, 2D] (chunk halves side by
                # side) over the full chunks; odd leftover as a 1 MB single.
                halves = {}  # chunk k -> (tile, col offset, rows)
                k = 0
                while k < nk:
                    if k + 1 < nk:
                        pair = tpool.tile([PCHUNK, 2 * D], f32, name="pair", tag="t")
                        src = feat[s, k * PCHUNK : (k + 2) * PCHUNK, :].rearrange(
                            "(c p) d -> p c d", p=PCHUNK
                        )
                        dst = pair[:].rearrange("p (c d) -> p c d", c=2)
                        dma_engines[n_dma % 2].dma_start(out=dst, in_=src)
                        halves[k] = (pair, 0, PCHUNK)
                        halves[k + 1] = (pair, D, PCHUNK)
                        k += 2
                    else:
                        single = tpool.tile([PCHUNK, D], f32, name="single", tag="t")
                        dma_engines[n_dma % 2].dma_start(
                            out=single[:], in_=feat[s, k * PCHUNK : (k + 1) * PCHUNK, :]
                        )
                        halves[k] = (single, 0, PCHUNK)
                        k += 1
                    n_dma += 1

                # VectorE path: full-on-every-core chunks, plain fp32 adds.
                for k in range(nu):
                    tile, off, _ = halves[k]
                    if k == 0:
                        nc.vector.tensor_copy(out=acc[:], in_=tile[:, off : off + D])
                    else:
                        nc.vector.tensor_add(
                            out=acc[:], in0=acc[:], in1=tile[:, off : off + D]
                        )

                # PE path: ragged chunks, per-core mask/len weights.
                n_mm = (nk - nu) + (1 if nu > 0 else 0)  # accumulation group size
                mm_i = 0
                for k in range(nu, nk):
                    tile, off, rows = halves[k]
                    col = s * MAXK + k
                    for j in range(D // NTILE):
                        nc.tensor.matmul(
                            psum_half[j][0:1, (j % 2) * NTILE : (j % 2 + 1) * NTILE],
                            mask_tile[0:rows, col : col + 1],
                            tile[0:rows, off + j * NTILE : off + (j + 1) * NTILE],
                            start=(mm_i == 0),
                            stop=(mm_i == n_mm - 1),
                        )
                    mm_i += 1

                # Cross-partition reduce of the DVE accumulator: 1/len column.
                if nu > 0:
                    col = SLOTS * MAXK + s
                    for j in range(D // NTILE):
                        nc.tensor.matmul(
                            psum_half[j][0:1, (j % 2) * NTILE : (j % 2 + 1) * NTILE],
                            mask_tile[:, col : col + 1],
                            acc[:, j * NTILE : (j + 1) * NTILE],
                            start=(mm_i == 0),
                            stop=True,
                        )

                # DVE (not ACT) for the PSUM->SBUF copy: the ACT sequencer
                # issues half the loads, and a copy queued behind a stalled
                # DMA issue would delay the PSUM release and stall the PE.
                out_t = opool.tile([1, D], f32)
                nc.vector.tensor_copy(out=out_t[:, 0 : D // 2], in_=psum_a[:])
                nc.vector.tensor_copy(out=out_t[:, D // 2 : D], in_=psum_b[:])
                nc.sync.dma_start(out=out[s : s + 1, :], in_=out_t[:])
    nc.finalize()
    return nc


def kernel(features, lengths):
    global LAST_RESULTS
    features = np.ascontiguousarray(features, dtype=np.float32)
    lengths = np.ascontiguousarray(lengths, dtype=np.int32)
    eff = np.where(lengths > 0, lengths, L).astype(np.int64)

    cores, slot_rows, slot_uniform = _plan(eff)
    key = (slot_rows, slot_uniform, TILE_BUFS)
    if key not in _PROGRAM_CACHE:
        _PROGRAM_CACHE[key] = _build_program(slot_rows, slot_uniform)
    nc = _PROGRAM_CACHE[key]

    in_maps = []
    for c in range(NCORES):
        perm = cores[c]
        maskt = np.zeros((PCHUNK, MCOLS), dtype=np.float32)
        for s, b in enumerate(perm):
            e = int(eff[b])
            inv = np.float32(1.0 / e)
            for k in range(slot_uniform[s], len(slot_rows[s])):
                lo = k * PCHUNK
                n_valid = min(max(e - lo, 0), PCHUNK)
                if n_valid > 0:
                    maskt[:n_valid, s * MAXK + k] = inv
            maskt[:, SLOTS * MAXK + s] = inv
        in_maps.append({"features": features[perm], "maskt": maskt})

    trace = os.environ.get("KERNEL_TRACE", "0") == "1"
    LAST_RESULTS = run_bass_kernel_spmd(
        nc,
        in_maps,
        core_ids=list(range(NCORES)),
        trace=trace,
        trace_cores=[0] if trace else None,
    )

    out = np.empty((B, D), dtype=np.float32)
    for c in range(NCORES):
        out[np.asarray(cores[c])] = LAST_RESULTS.results[c]["out"]
    return out



# revision 2
# speedup vs baseline: 1.0991x; 1.0991x over previous
"""Variable-length average pooling (prefix mean over seq axis) on 8 trn2 cores.

Strategy (pure data parallelism over batch, packed row stream):
  - eff_len[b] = lengths[b] if >0 else L.  pooled[b] = sum_{l<eff} x[b,l,:] / eff.
  - Sort batches by eff_len desc, snake-assign 16 per core; on the host, pack
    ONLY the valid rows of each core's 16 batches into one contiguous stream
    [R_c, 2048], zero-padded to NKC*128 rows (NKC = max_c ceil(R_c/128)).
    No per-slot round-up waste: 68 chunks/core here vs 78 for chunk-per-slot
    (-13% DMA bytes, the memory-bound floor).
  - One SPMD Bass program shared by all 8 cores; per-core raggedness lives
    entirely in a host-built weight tensor:
      * chunk k whose 128 rows belong to ONE slot on EVERY core ("interior")
        is summed on the VectorE into an SBUF accumulator (fp32 adds), later
        folded into PSUM by one matmul with a [128,16] column-select weight
        (col s = 1/eff_s).
      * every other chunk ("mixed": slot boundaries, tiny tail slots, pad)
        goes to the PE as psum[16,512] += wmat_k[128,16].T @ tile[128,512];
        wmat_k[p,s] = (row p owned by slot s on this core)/eff_s, pad rows 0.
    All matmuls accumulate into a single PSUM region [16,2048] (4 banks, one
    accumulation group per 512-col bank), so there is no per-slot PSUM
    recycling, no PE serialization on PSUM, and a ~2us tail: one [16,2048]
    PSUM->SBUF copy + one 128KB output DMA.
  - Chunks are fetched full-width (128 rows) in 2 MB pairs, alternating the
    two HWDGE rings (SP/ACT): a single ring caps at ~318 GB/s vs ~390-400 for
    the pair; partial-partition DMAs would pile onto the low SDMA engines.
  - Engine budget per core: DMA ~71 MB (~180us at ~400 GB/s), DVE ~90us,
    PE ~34 matmul groups ~80us: memory-bound with consumers at ~half load,
    so the load rings should never stall on buffer-reuse waits.
"""

import os

import numpy as np

import concourse.bacc as bacc
import concourse.mybir as mybir
from concourse.tile import TileContext
from concourse.bass_utils import run_bass_kernel_spmd

B, L, D = 128, 1024, 2048
NCORES = 8
SLOTS = B // NCORES  # 16
P = 128              # rows per chunk (partition dim)
NTILE = 512          # matmul free dim (one PSUM bank of fp32)

TILE_BUFS = int(os.environ.get("TILE_BUFS", "10"))

LAST_RESULTS = None  # BassKernelResults of the most recent device run


def _plan(eff):
    """Snake-assign sorted batches to cores and derive the shared chunk plan.

    Returns (cores[c][s] -> batch idx, offs[c][s] -> packed row offset,
    plan_key) where plan_key is the core-independent program structure:
    (NKC, chunk_class, interior, reduce_after, pe_idx, reduce_idx, n_pe,
    n_red)."""
    order = np.argsort(-eff, kind="stable")
    cores = [[] for _ in range(NCORES)]
    for i, idx in enumerate(order):
        blk, pos = divmod(i, NCORES)
        c = pos if blk % 2 == 0 else NCORES - 1 - pos
        cores[c].append(int(idx))
    offs = np.zeros((NCORES, SLOTS + 1), dtype=np.int64)
    for c in range(NCORES):
        offs[c, 1:] = np.cumsum([eff[b] for b in cores[c]])
    NKC = int(-(-offs[:, -1].max() // P))
    owner = np.full((NCORES, NKC * P), -1, dtype=np.int32)
    for c in range(NCORES):
        for s in range(SLOTS):
            owner[c, offs[c, s] : offs[c, s + 1]] = s

    chunk_class = []  # per k: ('dve', s) or ('pe', None)
    interior = {}     # slot -> list of interior chunk ks (contiguous range)
    for k in range(NKC):
        u = np.unique(owner[:, k * P : (k + 1) * P])
        if len(u) == 1 and u[0] >= 0:
            s = int(u[0])
            chunk_class.append(("dve", s))
            interior.setdefault(s, []).append(k)
        else:
            chunk_class.append(("pe", None))

    n_pe = 0
    pe_idx = {}
    for k, (t, _) in enumerate(chunk_class):
        if t == "pe":
            pe_idx[k] = n_pe
            n_pe += 1
    reduce_after = {}  # chunk k -> slots whose acc-reduce is emitted after k
    reduce_idx = {}
    n_red = 0
    for k in range(NKC):
        for s, ks in interior.items():
            if ks[-1] == k:
                reduce_after.setdefault(k, []).append(s)
                reduce_idx[s] = n_red
                n_red += 1
    plan = (
        NKC,
        tuple(chunk_class),
        {s: tuple(v) for s, v in interior.items()},
        {k: tuple(v) for k, v in reduce_after.items()},
        pe_idx,
        reduce_idx,
        n_pe,
        n_red,
    )
    return cores, offs, owner, plan


def _plan_cache_key(plan):
    NKC, chunk_class, interior, reduce_after, _, _, n_pe, n_red = plan
    return (
        NKC,
        chunk_class,
        tuple(sorted(interior.items())),
        tuple(sorted(reduce_after.items())),
        n_pe,
        n_red,
        TILE_BUFS,
    )


_PROGRAM_CACHE = {}


def _build_program(plan):
    NKC, chunk_class, interior, reduce_after, pe_idx, reduce_idx, n_pe, n_red = plan
    WCOLS = SLOTS * (n_pe + n_red)
    first_interior = {s: ks[0] for s, ks in interior.items()}
    n_groups = n_pe + n_red  # matmul accumulation contributions per bank

    # Bacc (not raw Bass): its compile pass splits multi-sem waits and moves
    # matmul waits onto ldweights — walrus allows only 1 wait per instruction.
    nc = bacc.Bacc(None, target_bir_lowering=False)
    f32 = mybir.dt.float32
    feat = nc.dram_tensor("features", [NKC * P, D], f32, kind="ExternalInput")
    wmat = nc.dram_tensor("wmat", [P, WCOLS], f32, kind="ExternalInput")
    out = nc.dram_tensor("out", [SLOTS, D], f32, kind="ExternalOutput")

    with TileContext(nc) as tc:
        with (
            tc.tile_pool(name="w", bufs=1) as wpool,
            tc.tile_pool(name="tiles", bufs=TILE_BUFS) as tpool,
            tc.tile_pool(name="accs", bufs=2) as apool,
            tc.tile_pool(name="psum", bufs=1, space="PSUM") as ppool,
            tc.tile_pool(name="outs", bufs=1) as opool,
        ):
            w_tile = wpool.tile([P, WCOLS], f32)
            nc.sync.dma_start(out=w_tile[:], in_=wmat[:])
            psum_t = ppool.tile([SLOTS, D], f32)

            # Load chunks in 2 MB pairs [128, 2D] (chunk halves side by side),
            # alternating the two HWDGE rings; odd leftover as a 1 MB single.
            dma_engines = [nc.sync, nc.scalar]
            halves = {}  # chunk k -> (tile, col offset)
            n_dma = 0
            k = 0
            while k < NKC:
                if k + 1 < NKC:
                    pair = tpool.tile([P, 2 * D], f32, name="pair", tag="t")
                    src = feat[k * P : (k + 2) * P, :].rearrange(
                        "(c p) d -> p c d", p=P
                    )
                    dst = pair[:].rearrange("p (c d) -> p c d", c=2)
                    dma_engines[n_dma % 2].dma_start(out=dst, in_=src)
                    halves[k] = (pair, 0)
                    halves[k + 1] = (pair, D)
                    k += 2
                else:
                    single = tpool.tile([P, D], f32, name="single", tag="t")
                    dma_engines[n_dma % 2].dma_start(
                        out=single[:], in_=feat[k * P : (k + 1) * P, :]
                    )
                    halves[k] = (single, 0)
                    k += 1
                n_dma += 1

            accs = {}
            g = 0  # accumulation-group contribution counter (per bank)

            def mm(wcol_off, rhs_tile, rhs_off):
                nonlocal g
                for j in range(D // NTILE):
                    nc.tensor.matmul(
                        psum_t[0:SLOTS, j * NTILE : (j + 1) * NTILE],
                        w_tile[:, wcol_off : wcol_off + SLOTS],
                        rhs_tile[:, rhs_off + j * NTILE : rhs_off + (j + 1) * NTILE],
                        start=(g == 0),
                        stop=(g == n_groups - 1),
                    )
                g += 1

            for k in range(NKC):
                t, s = chunk_class[k]
                tile, off = halves[k]
                if t == "dve":
                    if k == first_interior[s]:
                        accs[s] = apool.tile([P, D], f32, name="acc", tag="acc")
                        nc.vector.tensor_copy(
                            out=accs[s][:], in_=tile[:, off : off + D]
                        )
                    else:
                        nc.vector.tensor_add(
                            out=accs[s][:], in0=accs[s][:], in1=tile[:, off : off + D]
                        )
                else:
                    mm(SLOTS * pe_idx[k], tile, off)
                for s2 in reduce_after.get(k, ()):
                    mm(SLOTS * (n_pe + reduce_idx[s2]), accs[s2], 0)

            # DVE (not ACT) for the PSUM->SBUF copy: the ACT sequencer issues
            # half the loads; a copy queued behind a stalled DMA issue would
            # delay the output.
            out_t = opool.tile([SLOTS, D], f32)
            nc.vector.tensor_copy(out=out_t[:], in_=psum_t[:])
            nc.sync.dma_start(out=out[:], in_=out_t[:])
    nc.finalize()
    return nc


def kernel(features, lengths):
    global LAST_RESULTS
    features = np.ascontiguousarray(features, dtype=np.float32)
    lengths = np.ascontiguousarray(lengths, dtype=np.int32)
    eff = np.where(lengths > 0, lengths, L).astype(np.int64)

    cores, offs, owner, plan = _plan(eff)
    NKC, chunk_class, interior, reduce_after, pe_idx, reduce_idx, n_pe, n_red = plan
    WCOLS = SLOTS * (n_pe + n_red)

    key = _plan_cache_key(plan)
    if key not in _PROGRAM_CACHE:
        _PROGRAM_CACHE[key] = _build_program(plan)
    nc = _PROGRAM_CACHE[key]

    in_maps = []
    for c in range(NCORES):
        perm = cores[c]
        inv = 1.0 / eff[perm].astype(np.float32)
        packed = np.zeros((NKC * P, D), dtype=np.float32)
        for s, b in enumerate(perm):
            packed[offs[c, s] : offs[c, s + 1]] = features[b, : eff[b]]
        wmat = np.zeros((P, WCOLS), dtype=np.float32)
        own_c = owner[c].reshape(NKC, P)
        for k, (t, _) in enumerate(chunk_class):
            if t == "pe":
                o = own_c[k]
                valid = o >= 0
                wmat[valid, SLOTS * pe_idx[k] + o[valid]] = inv[o[valid]]
        for s, ridx in reduce_idx.items():
            wmat[:, SLOTS * (n_pe + ridx) + s] = inv[s]
        in_maps.append({"features": packed, "wmat": wmat})

    trace = os.environ.get("KERNEL_TRACE", "0") == "1"
    LAST_RESULTS = run_bass_kernel_spmd(
        nc,
        in_maps,
        core_ids=list(range(NCORES)),
        trace=trace,
        trace_cores=[0] if trace else None,
    )

    out = np.empty((B, D), dtype=np.float32)
    for c in range(NCORES):
        out[np.asarray(cores[c])] = LAST_RESULTS.results[c]["out"]
    return out


# revision 6
# speedup vs baseline: 1.1886x; 1.0814x over previous
"""Variable-length average pooling (prefix mean over seq axis) on 8 trn2 cores.

Strategy (pure data parallelism over batch, packed row stream):
  - eff_len[b] = lengths[b] if >0 else L.  pooled[b] = sum_{l<eff} x[b,l,:] / eff.
  - Sort batches by eff_len desc, snake-assign 16 per core; on the host, pack
    ONLY the valid rows of each core's 16 batches into one contiguous stream
    [R_c, 2048], zero-padded to NKC*128 rows (NKC = max_c ceil(R_c/128)).
    No per-slot round-up waste: 68 chunks/core here vs 78 for chunk-per-slot
    (-13% DMA bytes, the memory-bound floor).
  - One SPMD Bass program shared by all 8 cores; per-core raggedness lives
    entirely in a host-built weight tensor: every 128-row chunk k feeds
      psum[16,512j] += wmat_k[128,16].T @ tile[128,512j],   j = 0..3
    where wmat_k[p,s] = (row p owned by slot s on this core)/eff_s and pad
    rows are all-zero.  All 68 chunk-matmuls accumulate into a single PSUM
    region [16,2048] (4 banks, one accumulation group per bank): no per-slot
    PSUM recycling, no inter-chunk dependencies at all beyond the loads.
  - Tensors/tiles are declared float32r (same bytes as fp32): single-pass
    fp32 on the PE at 1 cycle/row for free dim >= 256, vs 4 cycles/row for
    exact fp32 (2 half-rate passes).  PE per chunk ~1us vs DMA ~2.5us, so
    the PE trails the loads by at most one chunk; fp32r's reduced multiplier
    precision (measured ~1e-4 rel) is irrelevant at the 2e-2 gate (PSUM
    accumulates fp32).
  - Chunks are fetched full-width (128 rows) in 2 MB pairs, round-robin over
    THREE DMA queues (SP/ACT/POOL sequencers): one HWDGE ring measured ~318
    GB/s, two ~390; the third is free since Pool issues nothing else.
  - Tail is ~3us: last matmul group -> one [16,2048] PSUM->SBUF copy (DVE)
    -> one 128 KB output DMA.
"""

import os

import numpy as np

import concourse.bacc as bacc
import concourse.mybir as mybir
from concourse.tile import TileContext
from concourse.bass_utils import run_bass_kernel_spmd

B, L, D = 128, 1024, 2048
NCORES = 8
SLOTS = B // NCORES  # 16
P = 128              # rows per chunk (partition dim)
NTILE = 512          # matmul free dim (one PSUM bank of fp32)

TILE_BUFS = int(os.environ.get("TILE_BUFS", "10"))
N_RINGS = int(os.environ.get("N_RINGS", "3"))
FP32R = os.environ.get("FP32R", "1") == "1"

LAST_RESULTS = None  # BassKernelResults of the most recent device run


def _plan(eff):
    """Snake-assign sorted batches to cores; derive packed offsets and NKC."""
    order = np.argsort(-eff, kind="stable")
    cores = [[] for _ in range(NCORES)]
    for i, idx in enumerate(order):
        blk, pos = divmod(i, NCORES)
        c = pos if blk % 2 == 0 else NCORES - 1 - pos
        cores[c].append(int(idx))
    offs = np.zeros((NCORES, SLOTS + 1), dtype=np.int64)
    for c in range(NCORES):
        offs[c, 1:] = np.cumsum([eff[b] for b in cores[c]])
    NKC = int(-(-offs[:, -1].max() // P))
    return cores, offs, NKC


_PROGRAM_CACHE = {}


def _build_program(NKC):
    # Bacc (not raw Bass): its compile pass splits multi-sem waits and moves
    # matmul waits onto ldweights — walrus allows only 1 wait per instruction.
    nc = bacc.Bacc(None, target_bir_lowering=False)
    f32 = mybir.dt.float32
    # float32r: same 4 bytes as fp32, but the PE runs a single full-rate pass
    # (1 cycle/row at free dim >= 512) instead of exact-fp32's 2 half-rate
    # passes.  Declared natively on the DRAM tensors and SBUF tiles (a
    # .bitcast() view fails walrus codegen).  Measured rel err ~1e-4.
    td = mybir.dt.float32r if FP32R else f32
    feat = nc.dram_tensor("features", [NKC * P, D], td, kind="ExternalInput")
    wmat = nc.dram_tensor("wmat", [P, SLOTS * NKC], td, kind="ExternalInput")
    out = nc.dram_tensor("out", [SLOTS, D], f32, kind="ExternalOutput")

    with TileContext(nc) as tc:
        with (
            tc.tile_pool(name="w", bufs=1) as wpool,
            tc.tile_pool(name="tiles", bufs=TILE_BUFS) as tpool,
            tc.tile_pool(name="psum", bufs=1, space="PSUM") as ppool,
            tc.tile_pool(name="outs", bufs=1) as opool,
        ):
            dma_engines = [nc.sync, nc.scalar, nc.gpsimd][:N_RINGS]
            w_tile = wpool.tile([P, SLOTS * NKC], td)
            dma_engines[-1].dma_start(out=w_tile[:], in_=wmat[:])
            psum_t = ppool.tile([SLOTS, D], f32)

            # Load chunks in 2 MB pairs [128, 2D] (chunk halves side by side),
            # round-robin over the DMA queues; odd leftover as a 1 MB single.
            halves = {}  # chunk k -> (tile, col offset)
            n_dma = 0
            k = 0
            while k < NKC:
                if k + 1 < NKC:
                    pair = tpool.tile([P, 2 * D], td, name="pair", tag="t")
                    src = feat[k * P : (k + 2) * P, :].rearrange(
                        "(c p) d -> p c d", p=P
                    )
                    dst = pair[:].rearrange("p (c d) -> p c d", c=2)
                    dma_engines[n_dma % len(dma_engines)].dma_start(out=dst, in_=src)
                    halves[k] = (pair, 0)
                    halves[k + 1] = (pair, D)
                    k += 2
                else:
                    single = tpool.tile([P, D], td, name="single", tag="t")
                    dma_engines[n_dma % len(dma_engines)].dma_start(
                        out=single[:], in_=feat[k * P : (k + 1) * P, :]
                    )
                    halves[k] = (single, 0)
                    k += 1
                n_dma += 1

            for k in range(NKC):
                tile, off = halves[k]
                for j in range(D // NTILE):
                    nc.tensor.matmul(
                        psum_t[0:SLOTS, j * NTILE : (j + 1) * NTILE],
                        w_tile[:, SLOTS * k : SLOTS * (k + 1)],
                        tile[:, off + j * NTILE : off + (j + 1) * NTILE],
                        start=(k == 0),
                        stop=(k == NKC - 1),
                    )

            # DVE for the PSUM->SBUF copy: the DMA-issuing sequencers are
            # busy draining loads; DVE is idle.
            out_t = opool.tile([SLOTS, D], f32)
            nc.vector.tensor_copy(out=out_t[:], in_=psum_t[:])
            dma_engines[-1].dma_start(out=out[:], in_=out_t[:])
    nc.finalize()
    return nc


def kernel(features, lengths):
    global LAST_RESULTS
    features = np.ascontiguousarray(features, dtype=np.float32)
    lengths = np.ascontiguousarray(lengths, dtype=np.int32)
    eff = np.where(lengths > 0, lengths, L).astype(np.int64)

    cores, offs, NKC = _plan(eff)
    key = (NKC, TILE_BUFS, N_RINGS, FP32R)
    if key not in _PROGRAM_CACHE:
        _PROGRAM_CACHE[key] = _build_program(NKC)
    nc = _PROGRAM_CACHE[key]

    in_maps = []
    rows = np.arange(NKC * P)
    for c in range(NCORES):
        perm = cores[c]
        inv = 1.0 / eff[perm].astype(np.float32)
        packed = np.zeros((NKC * P, D), dtype=np.float32)
        owner = np.full(NKC * P, -1, dtype=np.int64)
        for s, b in enumerate(perm):
            packed[offs[c, s] : offs[c, s + 1]] = features[b, : eff[b]]
            owner[offs[c, s] : offs[c, s + 1]] = s
        # wmat[p, 16k + s] = inv[s] iff row 128k+p belongs to slot s
        wmat = np.zeros((P, SLOTS * NKC), dtype=np.float32)
        valid = owner >= 0
        k_of, p_of = np.divmod(rows[valid], P)
        wmat[p_of, SLOTS * k_of + owner[valid]] = inv[owner[valid]]
        in_maps.append({"features": packed, "wmat": wmat})

    trace = os.environ.get("KERNEL_TRACE", "0") == "1"
    LAST_RESULTS = run_bass_kernel_spmd(
        nc,
        in_maps,
        core_ids=list(range(NCORES)),
        trace=trace,
        trace_cores=[0] if trace else None,
    )

    out = np.empty((B, D), dtype=np.float32)
    for c in range(NCORES):
        out[np.asarray(cores[c])] = LAST_RESULTS.results[c]["out"]
    return out
